# revision 44
# baseline (speedup 1.0000x reference)
"""Trainium2 Bass kernel for nn_DigitalTwinLoss.

Computes, over the full batch B:
  state_loss = sum(mask*(pred-target)^2) / (sum(mask)+eps)
  survival_loss = -mean_{e,b}[ ll(e,b) ],  ll = log_surv[idx] + ind*log_p[idx]
  total = state_loss + survival_loss

Data-parallel over the batch axis across 8 NeuronCores. Each core streams its
shard once and reduces to three per-partition partial sums; the host combines.

Key rewrite (gather-free): with bounds_k = 0.5*(k+1),
  idx      = sum_k [t > bounds_k]  (bounds_19 := +inf, so idx in [0,19], exact)
  log_surv[idx] = sum_k [t > bounds_k] * log1m_k      (the "lt" mask)
  ind*log_p[idx] = sum_k [idx' == k] * logp_k,  idx' = idx + 100*(1-ind)
so each (e,b) row needs two compares, one masked-sum; all exact in f32.
"""
import sys

sys.path.insert(0, "/opt/trn_rl_repo")

import numpy as np

import concourse.bacc as bacc
import concourse.bass as bass
import concourse.tile as tile
from concourse import mybir

B, T, E, K = 262144, 128, 5, 20
NCORES = 8
BC = B // NCORES  # rows per core
EPS = 1e-8

F32 = mybir.dt.float32
BF16 = mybir.dt.bfloat16
OP = mybir.AluOpType
AF = mybir.ActivationFunctionType


def build_nc(bc=BC, gb=16, g=128, reps=1, parts="all"):
    """Build the per-core SPMD program. bc: batch rows per core.

    reps>1 wraps the whole body in a For_i loop — used only for timing
    (per-call dispatch overhead >> kernel time on this setup)."""
    sw = gb * T                # state free width per partition
    nst = bc // (128 * gb)     # state tiles per tensor
    nhz_e = bc // (128 * g)    # hazard tiles per event
    nhz = nhz_e * E

    nc = bacc.Bacc()
    sp = nc.dram_tensor("sp", [bc, T], F32, kind="ExternalInput")
    st = nc.dram_tensor("st", [bc, T], F32, kind="ExternalInput")
    sm = nc.dram_tensor("sm", [bc, T], F32, kind="ExternalInput")
    hz = nc.dram_tensor("hz", [E, bc, K], F32, kind="ExternalInput")
    evt = nc.dram_tensor("evt", [E, bc], F32, kind="ExternalInput")
    evi = nc.dram_tensor("evi", [E, bc], F32, kind="ExternalInput")
    out = nc.dram_tensor("out", [128, 3], F32, kind="ExternalOutput")

    bnd_np = np.linspace(0.0, 10.0, K + 1, dtype=np.float32)[1:].copy()
    bnd_np[K - 1] = 1e30  # [t > bounds_19] always false -> idx <= 19
    kidx_np = np.arange(K, dtype=np.float32)
    consts_np = np.broadcast_to(
        np.stack([bnd_np, kidx_np])[None, :, :], (128, 2, K)
    ).copy()
    consts_dram = nc.inline_tensor(consts_np, name="consts")

    sp_t = sp.rearrange("(n p b) t -> n p (b t)", p=128, b=gb)
    st_t = st.rearrange("(n p b) t -> n p (b t)", p=128, b=gb)
    sm_t = sm.rearrange("(n p b) t -> n p (b t)", p=128, b=gb)
    hz_t = hz.rearrange("e (n p b) k -> e n p b k", p=128, b=g)
    # batched event loads: one DMA per j covering all 5 events
    evt_t = evt.rearrange("e (n p b) -> n p e b", p=128, b=g)
    evi_t = evi.rearrange("e (n p b) -> n p e b", p=128, b=g)

    with tile.TileContext(nc) as tc:
        with (
            tc.tile_pool(name="stin", bufs=3) as stin,
            tc.tile_pool(name="st16", bufs=2) as st16,
            tc.tile_pool(name="hzin", bufs=2) as hzin,
            tc.tile_pool(name="hzwork", bufs=3) as hzwork,
            tc.tile_pool(name="small", bufs=4) as small,
            tc.tile_pool(name="evp", bufs=1) as evp,
            tc.tile_pool(name="persist", bufs=1) as persist,
            tc.tile_pool(name="psum", bufs=1, space="PSUM") as psum,
        ):
            num_slots = persist.tile([128, nst], F32)
            den_slots = persist.tile([128, nst], F32)
            consts = persist.tile([128, 2, K], F32)
            nc.sync.dma_start(out=consts[:], in_=consts_dram[:])
            bnd_b = consts[:, 0:1, :]   # [128,1,K]
            kidx_b = consts[:, 1:2, :]  # [128,1,K]
            eps_b = persist.tile([128, 1], F32)
            one_eps_b = persist.tile([128, 1], F32)
            ones16 = persist.tile([128, 1], BF16)
            nc.vector.memset(eps_b[:], EPS)
            nc.vector.memset(one_eps_b[:], np.float32(1.0 + EPS))
            nc.vector.memset(ones16[:], 1.0)
            # PE accumulates all ll contributions as PSUM column sums
            llp = psum.tile([1, 512], F32)
            n_mm = (2 * g * K) // 512  # matmul chunks per hazard tile

            # spread big loads over the two HW DGE queues (SP, ACT);
            # gpsimd's Q7 cores are used for compute offload instead
            dma_engines = [nc.sync, nc.scalar]
            n_eng = len(dma_engines)

            def state_tile(i):
                a = stin.tile([128, sw], F32, tag="a")
                bt = stin.tile([128, sw], F32, tag="b")
                m = stin.tile([128, sw], F32, tag="m")
                d16 = st16.tile([128, sw], BF16, tag="d16")
                m16 = st16.tile([128, sw], BF16, tag="m16")
                dma_engines[i % n_eng].dma_start(out=a[:], in_=sp_t[i])
                dma_engines[(i + 1) % n_eng].dma_start(out=bt[:], in_=st_t[i])
                dma_engines[i % n_eng].dma_start(out=m[:], in_=sm_t[i])
                # d16 = (pred-target) in bf16; mask cast on ACT; mul in bf16
                nc.vector.tensor_sub(d16[:], a[:], bt[:])
                nc.scalar.activation(
                    out=m16[:], in_=m[:], func=AF.Copy,
                    accum_out=den_slots[:, i : i + 1],
                )
                nc.vector.tensor_mul(d16[:], d16[:], m16[:])
                # sum(mask*d^2) == sum((mask*d)^2) since mask is 0/1
                nc.scalar.activation(
                    out=d16[:], in_=d16[:], func=AF.Square,
                    accum_out=num_slots[:, i : i + 1],
                )

            ev_tiles = {}

            def load_events(j):
                t5 = evp.tile([128, E, g, 1], F32, tag=f"t5_{j}")
                i5 = evp.tile([128, E, g, 1], F32, tag=f"i5_{j}")
                nc.sync.dma_start(out=t5[:, :, :, 0], in_=evt_t[j])
                nc.sync.dma_start(out=i5[:, :, :, 0], in_=evi_t[j])
                ev_tiles[j] = (t5, i5)

            def hazard_tile(e, j, s):
                L = hzin.tile([128, g, K], F32, tag="L")
                P16 = hzin.tile([128, g, K], BF16, tag="P16")
                ii = small.tile([128, g, 1], F32, tag="i")
                idx = small.tile([128, g, 1], F32, tag="idx")
                SP = hzwork.tile([128, g, 2, K], BF16, tag="SP")
                WW = hzwork.tile([128, g, 2, K], BF16, tag="WW")
                t5, i5 = ev_tiles[j]
                tt = t5[:, e]
                dma_engines[s % n_eng].dma_start(out=L[:], in_=hz_t[e, j])
                # ACT: p = sigmoid(L); SP0 = log(1-p+eps); SP1 = log(p+eps)
                nc.scalar.activation(out=P16[:], in_=L[:], func=AF.Sigmoid)
                nc.scalar.activation(
                    out=SP[:, :, 0, :], in_=P16[:], func=AF.Ln,
                    scale=-1.0, bias=one_eps_b[:],
                )
                nc.scalar.activation(
                    out=SP[:, :, 1, :], in_=P16[:], func=AF.Ln,
                    scale=1.0, bias=eps_b[:],
                )
                # DVE: lt mask, idx, one-hot eq (with indicator folded in)
                nc.vector.tensor_tensor(
                    WW[:, :, 0, :],
                    tt.to_broadcast((128, g, K)),
                    bnd_b.to_broadcast((128, g, K)),
                    op=OP.is_gt,
                )
                nc.vector.tensor_reduce(
                    out=idx[:, :, 0], in_=WW[:, :, 0, :],
                    axis=mybir.AxisListType.X, op=OP.add,
                )
                # ii <- 100 - 100*ind ; idx' = idx + ii
                nc.vector.tensor_scalar(
                    out=ii[:], in0=i5[:, e], scalar1=-100.0, scalar2=100.0,
                    op0=OP.mult, op1=OP.add,
                )
                nc.vector.tensor_add(idx[:], idx[:], ii[:])
                nc.vector.tensor_tensor(
                    WW[:, :, 1, :],
                    idx[:].to_broadcast((128, g, K)),
                    kidx_b.to_broadcast((128, g, K)),
                    op=OP.is_equal,
                )
                # ll contribution: sum(WW * SP); product on DVE, partition
                # sums on the (otherwise idle) TensorEngine into PSUM
                # (TensorTensorReduce faults on this toolchain)
                nc.vector.tensor_mul(WW[:], WW[:], SP[:])
                wwf = WW[:].rearrange("p g t k -> p (g t k)")
                for c in range(n_mm):
                    nc.tensor.matmul(
                        llp[:],
                        ones16[:],
                        wwf[:, c * 512 : (c + 1) * 512],
                        start=(s == 0 and c == 0),
                        stop=(s == nhz - 1 and c == n_mm - 1),
                    )

            # interleave state and hazard tiles so DMA/ACT/DVE overlap;
            # front-load hazard tiles (long DVE chains) and end on state
            # tiles (short chains) to shrink the kernel tail
            def hz_job(hi):
                return ("h", (hi // nhz_e, hi % nhz_e, hi))

            merged = []
            si = hi = 0
            lead = min(2, nhz)
            while hi < lead:
                merged.append(hz_job(hi)); hi += 1
            while si < nst or hi < nhz:
                rem_s, rem_h = nst - si, nhz - hi
                if rem_s * (nhz - lead) >= rem_h * nst and si < nst:
                    merged.append(("s", si)); si += 1
                elif hi < nhz:
                    merged.append(hz_job(hi)); hi += 1
            if parts == "state":
                merged = [jb for jb in merged if jb[0] == "s"]
            elif parts == "hazard":
                merged = [jb for jb in merged if jb[0] == "h"]
            res = persist.tile([128, 3], F32)

            if parts == "hazard":
                nc.vector.memset(num_slots[:], 0.0)
                nc.vector.memset(den_slots[:], 0.0)

            def body():
                if parts != "state":
                    for j in range(nhz_e):
                        load_events(j)
                for kind, arg in merged:
                    if kind == "s":
                        state_tile(arg)
                    else:
                        hazard_tile(*arg)
                nc.vector.memset(res[:, 2:3], 0.0)
                nc.vector.reduce_sum(
                    out=res[:, 0:1], in_=num_slots[:], axis=mybir.AxisListType.X
                )
                nc.vector.reduce_sum(
                    out=res[:, 1:2], in_=den_slots[:], axis=mybir.AxisListType.X
                )
                if parts != "state":
                    nc.vector.reduce_sum(
                        out=res[0:1, 2:3], in_=llp[:], axis=mybir.AxisListType.X
                    )
                nc.sync.dma_start(out=out[:], in_=res[:])

            if reps == 1:
                body()
            else:
                with tc.For_i(0, reps, 1):
                    body()

    nc.compile()
    return nc


_CACHE = {}


def _get_nc():
    if "nc" not in _CACHE:
        _CACHE["nc"] = build_nc()
    return _CACHE["nc"]


def make_in_maps(inputs):
    sp = np.asarray(inputs["state_pred"], dtype=np.float32)
    st = np.asarray(inputs["state_target"], dtype=np.float32)
    sm = np.asarray(inputs["state_mask"], dtype=np.float32)
    hz = np.asarray(inputs["hazard_logits"], dtype=np.float32)
    evt = np.asarray(inputs["event_times"], dtype=np.float32)
    evi = np.asarray(inputs["event_indicators"], dtype=np.float32)
    in_maps = []
    for c in range(NCORES):
        sl = slice(c * BC, (c + 1) * BC)
        in_maps.append(
            {
                "sp": np.ascontiguousarray(sp[sl]),
                "st": np.ascontiguousarray(st[sl]),
                "sm": np.ascontiguousarray(sm[sl]),
                "hz": np.ascontiguousarray(hz[:, sl, :]),
                "evt": np.ascontiguousarray(evt[sl].T),
                "evi": np.ascontiguousarray(evi[sl].T),
            }
        )
    return in_maps


def combine(parts):
    """parts: [ncores, 128, 3] partial sums -> scalar loss."""
    s = np.asarray(parts, dtype=np.float64).sum(axis=(0, 1))
    state_loss = s[0] / (s[1] + EPS)
    survival = -s[2] / (E * B)
    return np.asarray(state_loss + survival, dtype=np.float32)


def kernel(**inputs):
    from concourse.bass_utils import run_bass_kernel_spmd

    nc = _get_nc()
    in_maps = make_in_maps(inputs)
    res = run_bass_kernel_spmd(nc, in_maps, list(range(NCORES)))
    parts = np.stack([np.asarray(r["out"]) for r in res.results])
    return combine(parts)
